# revision 21
# baseline (speedup 1.0000x reference)
"""Hashed-weight MLP (1024-4096-4096-32000, batch 2048) on 8 TRN2 NeuronCores.

Problem: h = relu(x @ W0); h = relu(h @ W1); out = h @ W2, where each
W_l[i, j] = hw_l[(a_l*i + b_l*j + c_l) % N_l] is a virtual (ROBE-Z hashed)
weight gathered from a small parameter vector.

Approach (column-parallel tensor parallelism on all three layers):
  * Via the host-permuted table hw_bb[t] = hw[(b*t) % N] the virtual weight
    becomes row-contiguous: W[i, col] = hw_bb[shift + q*kk + r*c1 + col] with
    i = k*c1 + kk (q = b^-1 a, r = signed residue of q*k mod N). Weight tiles
    are DMAd STRAIGHT from the per-core slice into SBUF (no DRAM
    materialization): one 3-level strided DMA per block-aligned tile for
    L0/L2 (partitions in kk-outer permuted order, with the matching
    permutation applied to the rhs activation tiles), ~4 run DMAs per
    natural-order tile for L1.
  * Each core owns a 1/8 column shard of every layer; shard offsets are
    absorbed into the host-side slice (SPMD-uniform device program).
  * L2 keeps the whole 4096 x 2048 h2 activation resident in SBUF (loaded
    once after each AllGather chunk) and streams W2 tiles through a ring,
    so the big GEMM phase reads each byte from HBM exactly once.
  * Engines: scalar = weight-ladder DMA triggers only; sync = activation
    streams/stores; vector = ReLU + PSUM evacuation (bf16 cast); gpsimd =
    AllGather triggers. GEMMs are bf16 with fp32 PSUM (rel err ~4e-3).
"""
import sys
if "/opt/trn_rl_repo" not in sys.path:
    sys.path.insert(0, "/opt/trn_rl_repo")

import numpy as np
import ml_dtypes

import concourse.bass as bass
import concourse.bacc as bacc
import concourse.tile as tile
import concourse.mybir as mybir
from concourse.bass_utils import run_bass_kernel_spmd

N_CORES = 8
P = 128
NB = 512                      # batch tile (matmul moving dim)
BATCH = 2048
BT = BATCH // NB              # 4

LENS = [1024, 4096, 4096, 32000]
HASH_A = [9973, 10007, 10039]
HASH_B = [31013, 31019, 31039]
HASH_C = [557, 563, 569]
SIZES = [1048576, 1048576, 4194304]
JW = [512, 512, 4000]         # true per-core output shard width
WTOT = [512, 512, 4096]       # max ladder col offset (L2 incl. jg offsets)
TILE_H = [120, 128, 126]      # k-tile heights (L0/L2 block-aligned to k)

BF = mybir.dt.bfloat16
F32 = mybir.dt.float32


def _plan_layer(l):
    N = SIZES[l]; a, b, ch = HASH_A[l], HASH_B[l], HASH_C[l]
    binv = pow(b, -1, N)
    q = (binv * a) % N
    u0 = (binv * ch) % N
    in_dim = LENS[l]
    best = None
    for k in range(1, min(in_dim, 600) + 1):
        r = (q * k) % N
        if r > N // 2:
            r -= N
        C1 = -(-in_dim // k)
        extra = q * (k - 1) + abs(r) * (C1 - 1)
        if best is None or extra < best[0]:
            best = (extra, k, C1, r)
    _, k, C1, r = best
    shift = max(0, -r * (C1 - 1))
    m_ext = shift + q * (k - 1) + max(r, 0) * (C1 - 1) + WTOT[l] + 64
    return dict(N=N, b=b, q=q, u0=u0, k=k, r=r, shift=shift,
                m_ext=m_ext, in_dim=in_dim)


PLANS = [_plan_layer(l) for l in range(3)]
RG = [list(range(N_CORES))]


def _tiles_perm(k, rows_total, tile_h):
    """Block-aligned tiles; seg = (kk0, c1_0, kc, c1c, p0); within a seg the
    DMA streams kk-outer so partition p = p0 + kkrel*c1c + c1rel."""
    tiles = []
    i0 = 0
    while i0 < rows_total:
        h = min(tile_h, rows_total - i0)
        c1_0 = i0 // k
        segs = []
        full_c1 = h // k
        if full_c1:
            segs.append((0, c1_0, k, full_c1, 0))
        rem = h - full_c1 * k
        if rem:
            segs.append((0, c1_0 + full_c1, rem, 1, k * full_c1))
        tiles.append((i0, h, segs))
        i0 += h
    return tiles


def _tiles_nat(k, rows_total, tile_h):
    """Natural-order tiles, <=3 segs each: head run to the block boundary,
    a c1-outer rectangle of full blocks (legal only when r > 0), tail run.
    Natural row order on both the weight and rhs side."""
    tiles = []
    for i0 in range(0, rows_total, tile_h):
        h = min(tile_h, rows_total - i0)
        segs = []
        row = i0
        c1, kk = divmod(row, k)
        if kk:
            cnt = min(k - kk, h)
            segs.append((kk, c1, cnt, 1, row - i0))
            row += cnt
        nfull = (i0 + h - row) // k
        if nfull:
            # c1-outer rect: stream order == natural row order
            segs.append(("rect", row // k, k, nfull, row - i0))
            row += nfull * k
        if row < i0 + h:
            segs.append((0, row // k, i0 + h - row, 1, row - i0))
        tiles.append((i0, h, segs))
    return tiles


LT = [
    _tiles_perm(PLANS[0]["k"], 1024, TILE_H[0]),   # 9 tiles (8x120 + 64)
    _tiles_nat(PLANS[1]["k"], 4096, TILE_H[1]),    # 32 tiles x ~4 segs
    _tiles_perm(PLANS[2]["k"], 4096, TILE_H[2]),   # 33 tiles (32x126 + 64)
]
NKT = [len(t) for t in LT]
NJG2 = 8                      # L2 j-groups of width 512 (4 j-tiles of 128)


def _ladder_dmas(nc, eng, hb_t, l, wtile_ap, tile_idx, col0, w):
    """Emit ladder DMAs for weight tile tile_idx of layer l into SBUF tile."""
    pl = PLANS[l]
    q, r, shift = pl["q"], pl["r"], pl["shift"]
    _, h, segs = LT[l][tile_idx]
    for (kk0, c1_0, kc, c1c, p0) in segs:
        if kk0 == "rect":
            # natural-order full-block rect (c1 outer); requires r > 0
            src = bass.AP(hb_t, shift + r * c1_0 + col0,
                          [[r, c1c], [q, kc], [1, w]])
        elif c1c == 1:
            src = bass.AP(hb_t, shift + q * kk0 + r * c1_0 + col0,
                          [[q, kc], [1, w]])
        else:
            src = bass.AP(hb_t, shift + q * kk0 + r * c1_0 + col0,
                          [[q, kc], [r, c1c], [1, w]])
        eng.dma_start(out=wtile_ap[p0:p0 + kc * c1c, :], in_=src)


def _act_dmas(nc, eng, act_t, l, dst_ap, tile_idx, rowstride, col0, w):
    """Load activation rows matching weight tile tile_idx's partition order.
    act_t is a DRAM tensor whose rows are the layer's contraction rows."""
    pl = PLANS[l]
    k = pl["k"]
    _, h, segs = LT[l][tile_idx]
    for (kk0, c1_0, kc, c1c, p0) in segs:
        base = (k * c1_0 + kk0) * rowstride + col0
        if c1c == 1:
            src = bass.AP(act_t, base, [[rowstride, kc], [1, w]])
        else:
            src = bass.AP(act_t, base,
                          [[rowstride, kc], [k * rowstride, c1c], [1, w]])
        eng.dma_start(out=dst_ap[p0:p0 + kc * c1c, :], in_=src)


def build_nc():
    nc = bacc.Bacc("TRN2", target_bir_lowering=False, debug=False,
                   num_devices=N_CORES)

    xT_d = nc.dram_tensor("xT", [LENS[0], BATCH], BF, kind="ExternalInput").ap()
    # W0/W1 are host-materialized tile-major (wide rows -> 8KB descriptors);
    # only the big W2 still streams via the on-device hash ladder.
    w0m_d = nc.dram_tensor("w0m", [128, 4096], BF, kind="ExternalInput").ap()
    w1m_d = nc.dram_tensor("w1m", [4, 128, 4096], BF,
                           kind="ExternalInput").ap()
    hb2 = nc.dram_tensor("hb2", [PLANS[2]["m_ext"]], BF,
                         kind="ExternalInput").ap()
    h1c = [nc.dram_tensor(f"h1c{b}", [512, NB], BF).ap() for b in range(BT)]
    h1f = [nc.dram_tensor(f"h1f{b}", [4096, NB], BF, addr_space="Shared").ap()
           for b in range(BT)]
    h2c = [nc.dram_tensor(f"h2c{b}", [512, NB], BF).ap() for b in range(BT)]
    h2f = [nc.dram_tensor(f"h2f{b}", [4096, NB], BF, addr_space="Shared").ap()
           for b in range(BT)]
    out_d = nc.dram_tensor("outT", [4096, BATCH], BF, kind="ExternalOutput").ap()

    with tile.TileContext(nc) as tc, \
         tc.tile_pool(name="ps", bufs=8, space="PSUM") as psp, \
         tc.tile_pool(name="resid", bufs=1) as residp:

        # h2 stays fully SBUF-resident for L2 (132 KB/partition)
        h2res = [[residp.tile([LT[2][kt][1], NB], BF, name=f"h2r_{b}_{kt}")
                  for kt in range(NKT[2])] for b in range(BT)]

        def load_h2res(b):
            # on gpsimd (SWDGE): third DGE lane, keeps both HWDGE rings free
            with nc.allow_non_contiguous_dma(reason="perm act load"):
                for kt in range(NKT[2]):
                    _act_dmas(nc, nc.gpsimd, h2f[b].tensor, 2,
                              h2res[b][kt][:], kt, NB, 0, NB)

        # head of the first L2 slab: own (outer) pool so its ladders run
        # during L0/L1 without write-after-read waits on dying pools
        NHEAD = 12
        with nc.allow_non_contiguous_dma(reason="hash ladder"), \
             tc.tile_pool(name="jg0h", bufs=1) as jg0hp:
            jg0head = [jg0hp.tile([LT[2][kt][1], 512], BF, name=f"w2h{kt}")
                       for kt in range(NHEAD)]

            # w1sb lives below the L0 pools; its ladders prefetch during L0
            with tc.tile_pool(name="l1w", bufs=1) as l1wp:
                w1sb = [l1wp.tile([128, 4096], BF, name=f"w1sb{t}")
                        for t in range(4)]

                # ---------------- Layer 0 ----------------
                with tc.tile_pool(name="l0", bufs=1) as l0p, \
                     tc.tile_pool(name="l0x", bufs=14) as l0xp, \
                     tc.tile_pool(name="l0h", bufs=6) as l0hp:
                    w0sb = l0p.tile([128, 4096], BF, name="w0sb")
                    nc.scalar.dma_start(out=w0sb[:], in_=w0m_d[:, :])
                    for t in range(4):
                        eng = nc.sync if t % 2 == 0 else nc.scalar
                        eng.dma_start(out=w1sb[t][:], in_=w1m_d[t, :, :])
                    for kt in range(NHEAD):
                        eng = nc.scalar if kt % 2 == 0 else nc.sync
                        _ladder_dmas(nc, eng, hb2.tensor, 2,
                                     jg0head[kt][:], kt, 0, 512)

                    def load_x(b):
                        xsb = []
                        for kt in range(8):
                            xt = l0xp.tile([128, NB], BF, tag="x",
                                           name=f"x_{b}_{kt}")
                            nc.gpsimd.dma_start(
                                out=xt[:],
                                in_=xT_d[kt * P:(kt + 1) * P,
                                         b * NB:(b + 1) * NB])
                            xsb.append(xt)
                        return xsb

                    xq = [load_x(0), load_x(1)]
                    for b in range(BT):
                        for j in range(4):
                            ps = psp.tile([P, NB], F32, tag="ps",
                                          name=f"ps0_{b}_{j}")
                            for kt in range(8):
                                nc.tensor.matmul(
                                    out=ps[:],
                                    lhsT=w0sb[:, kt * 512 + j * P:
                                              kt * 512 + (j + 1) * P],
                                    rhs=xq[b][kt][:],
                                    start=(kt == 0), stop=(kt == 7))
                            hsb = l0hp.tile([P, NB], BF, tag="h1",
                                            name=f"h1_{b}_{j}")
                            nc.vector.tensor_scalar_max(hsb[:], ps[:], 0.0)
                            nc.gpsimd.dma_start(
                                out=h1c[b][j * P:(j + 1) * P, :], in_=hsb[:])
                        nc.gpsimd.collective_compute(
                            "AllGather", mybir.AluOpType.bypass,
                            replica_groups=RG,
                            ins=[h1c[b].opt()], outs=[h1f[b].opt()])
                        if b + 2 < BT:
                            xq.append(load_x(b + 2))

                # ---------------- Layer 1 ----------------
                with tc.tile_pool(name="l1r", bufs=16) as l1rp, \
                     tc.tile_pool(name="l1h", bufs=8) as l1hp:
                    for b in range(BT):
                        pss = [psp.tile([P, NB], F32, tag="ps",
                                        name=f"ps1_{b}_{j}")
                               for j in range(4)]
                        for kt in range(NKT[1]):
                            rhs = l1rp.tile([P, NB], BF, tag="l1rhs",
                                            name=f"l1r_{b}_{kt}")
                            reng = nc.sync if kt % 2 == 0 else nc.scalar
                            reng.dma_start(
                                out=rhs[:],
                                in_=h1f[b][kt * P:(kt + 1) * P, :])
                            for j in range(4):
                                co = (kt % 8) * 512
                                nc.tensor.matmul(
                                    out=pss[j][:],
                                    lhsT=w1sb[kt // 8][:, co + j * P:
                                                       co + (j + 1) * P],
                                    rhs=rhs[:],
                                    start=(kt == 0), stop=(kt == NKT[1] - 1))
                        for j in range(4):
                            hsb = l1hp.tile([P, NB], BF, tag="h2",
                                            name=f"h2_{b}_{j}")
                            nc.vector.tensor_scalar_max(hsb[:], pss[j][:], 0.0)
                            nc.gpsimd.dma_start(
                                out=h2c[b][j * P:(j + 1) * P, :], in_=hsb[:])
                        nc.gpsimd.collective_compute(
                            "AllGather", mybir.AluOpType.bypass,
                            replica_groups=RG,
                            ins=[h2c[b].opt()], outs=[h2f[b].opt()])
                        # residency loads for AGs that finished earlier;
                        # emitted after this b's stores+AG so they don't
                        # delay the store->AG chain on the gpsimd queue
                        if b >= 2:
                            load_h2res(b - 2)

                    load_h2res(2)
                    load_h2res(3)

            # ---------------- Layer 2 ----------------
            with tc.tile_pool(name="w2", bufs=56) as w2p, \
                 tc.tile_pool(name="l2o", bufs=6) as l2op:
                for jg in range(NJG2):
                    slab = []
                    for kt in range(NKT[2]):
                        if jg == 0 and kt < NHEAD:
                            slab.append(jg0head[kt])
                            continue
                        wt = w2p.tile([LT[2][kt][1], 512], BF, tag="w2t",
                                      name=f"w2_{jg}_{kt}")
                        weng = nc.scalar if kt % 2 == 0 else nc.sync
                        _ladder_dmas(nc, weng, hb2.tensor, 2, wt[:],
                                     kt, jg * 512, 512)
                        slab.append(wt)
                    for b in range(BT):
                        pss = [psp.tile([P, NB], F32, tag="ps",
                                        name=f"ps2_{jg}_{b}_{j}")
                               for j in range(4)]
                        for kt in range(NKT[2]):
                            for j in range(4):
                                nc.tensor.matmul(
                                    out=pss[j][:],
                                    lhsT=slab[kt][:, j * P:(j + 1) * P],
                                    rhs=h2res[b][kt][:],
                                    start=(kt == 0), stop=(kt == NKT[2] - 1))
                        for j in range(4):
                            osb = l2op.tile([P, NB], BF, tag="o",
                                            name=f"o_{jg}_{b}_{j}")
                            nc.vector.tensor_copy(out=osb[:], in_=pss[j][:])
                            nc.sync.dma_start(
                                out=out_d[jg * 512 + j * P:
                                          jg * 512 + (j + 1) * P,
                                          b * NB:(b + 1) * NB],
                                in_=osb[:])

    nc.compile()
    return nc


_NC_CACHE = None


def _get_nc():
    global _NC_CACHE
    if _NC_CACHE is None:
        _NC_CACHE = build_nc()
    return _NC_CACHE


def _prep_inputs(x, hw0, hw1, hw2):
    """Host prep: transpose x, build per-core periodic permuted-table slices."""
    x = np.asarray(x, np.float32)
    hws = [np.asarray(hw0, np.float32), np.asarray(hw1, np.float32),
           np.asarray(hw2, np.float32)]
    xT = np.ascontiguousarray(x.T).astype(ml_dtypes.bfloat16)

    # hb2 slice (device-side ladder source for W2)
    pl = PLANS[2]
    N2, b2 = pl["N"], pl["b"]
    t0 = pl["u0"] - pl["shift"]
    span = pl["m_ext"] + (N_CORES - 1) * JW[2]
    t = t0 + np.arange(span, dtype=np.int64)
    shared2 = hws[2][(b2 * t) % N2].astype(ml_dtypes.bfloat16)

    # host-materialized W0/W1 shards, tile-major wide layout
    def mat_w(l, c):
        i = np.arange(LENS[l], dtype=np.int64)[:, None]
        j = c * JW[l] + np.arange(JW[l], dtype=np.int64)[None, :]
        w = hws[l][(i * HASH_A[l] + j * HASH_B[l] + HASH_C[l]) % SIZES[l]]
        return w.astype(ml_dtypes.bfloat16)

    in_maps = []
    for c in range(N_CORES):
        w0 = mat_w(0, c)                     # [1024, 512]
        w0m = np.ascontiguousarray(
            w0.reshape(8, 128, 512).transpose(1, 0, 2).reshape(128, 4096))
        w1 = mat_w(1, c)                     # [4096, 512]
        w1m = np.ascontiguousarray(
            w1.reshape(4, 8, 128, 512).transpose(0, 2, 1, 3)
              .reshape(4, 128, 4096))
        in_maps.append({
            "xT": xT,
            "w0m": w0m,
            "w1m": w1m,
            "hb2": shared2[c * JW[2]: c * JW[2] + pl["m_ext"]],
        })
    return in_maps


def kernel(x, hw0, hw1, hw2, trace=False):
    nc = _get_nc()
    in_maps = _prep_inputs(x, hw0, hw1, hw2)
    res = run_bass_kernel_spmd(nc, in_maps, list(range(N_CORES)), trace=trace)
    outs = [np.asarray(res.results[c]["outT"][:JW[2], :])
            for c in range(N_CORES)]
    full = np.concatenate(outs, axis=0)         # [32000, 2048] bf16
    out = np.ascontiguousarray(full.T).astype(np.float32)
    kernel.last_results = res
    return out
